# revision 1
# baseline (speedup 1.0000x reference)
"""Trainium2 Bass kernel for GRNNTransformSimple (bottom-up binary-tree GRNN).

Computation (per jet): heap-layout complete binary tree, DEPTH=14.
  u_k   = relu(contents_k @ Wu + bu)                         (all nodes)
  emb_k = u_k                                                (leaves)
  emb_k = relu(hL @ Wh[:64] + hR @ Wh[64:128] + u_k @ Wh[128:] + bh)  (inner)
Output: root emb, [B, 64].

Mapping (8 NeuronCores, data-parallel over B=128 jets, 16 jets/core):
 - 2 jets packed per 128 SBUF partitions (jet A on partitions 0-63, jet B on
   64-127) with block-diagonal weights -> all engines run 128 partitions wide.
 - fc_u biases folded into the matmul via a constant-one input row (K=18).
 - fc_u uses 4-way PE row tiling (tile_position) since K=18 << 128.
 - The "irregular" child gather is regular for arange children: children of
   level-i node j are nodes 2j, 2j+1 of level i+1, i.e. a stride-2 column
   slice of the level-(i+1) embedding buffer.
 - Levels 9..0 are batched across all 8 jet-pairs of the core (pair-major
   columns) to amortize instruction overheads.
"""

import os
import sys

sys.path.insert(0, "/opt/trn_rl_repo")

import ml_dtypes
import numpy as np

DEPTH = 14
B = 128
F = 8
H = 64
N_NODES = 2**DEPTH - 1  # 16383
N_INNER = 2 ** (DEPTH - 1) - 1  # 8191
N_CORES = 8
JPC = 16  # jets per core
NPAIR = 8  # jet pairs per core

BF16 = ml_dtypes.bfloat16

# u_stream layout per pair (columns): levels 10,11,12 inner nodes in heap
# order, then all leaves in heap order.
UB10, UB11, UB12 = 0, 1024, 3072  # level bases inside u_stream
ULEAF = 7168
USTREAM = 15360  # 1024 + 2048 + 4096 + 8192
NGRP = 15  # 15 groups x 1024 cols
# u_top: levels 0..9, column order [level][pair][node]
UTOP_COLS = 8184  # 8 * 1023
UTOP_PAD = 8192


def _np_reference(contents, children, Wu, bu, Wh, bh):
    emb = None
    for i in range(DEPTH - 1, -1, -1):
        off, n = 2**i - 1, 2**i
        u = np.maximum(contents[:, off : off + n] @ Wu + bu, 0)
        if emb is None:
            emb = u
        else:
            ch = children[off : off + n] - 2 * off
            hL = emb[:, ch[:, 0]]
            hR = emb[:, ch[:, 1]]
            emb = np.maximum(
                hL @ Wh[:H] + hR @ Wh[H : 2 * H] + u @ Wh[2 * H :] + bh, 0
            )
    return emb.reshape(emb.shape[0], -1).astype(np.float32)


def _prep_core_inputs(contents):
    """contents: [16, 16383, 8] f32 for one core.
    Returns dict of per-core device input arrays."""
    c4 = np.zeros((NPAIR, 128, 4096), dtype=BF16)
    big_T = np.ascontiguousarray(
        np.transpose(contents[:, 1023:16383, :], (0, 2, 1))
    )  # [16, 8, 15360]
    for p in range(NPAIR):
        S = np.empty((18, USTREAM), dtype=np.float32)
        S[0:8] = big_T[2 * p]
        S[8] = 1.0
        S[9:17] = big_T[2 * p + 1]
        S[17] = 1.0
        Sb = S.astype(BF16)
        for g in range(NGRP):
            t = g % 4
            cc = 1024 * (g // 4)
            c4[p, 32 * t : 32 * t + 18, cc : cc + 1024] = Sb[
                :, 1024 * g : 1024 * (g + 1)
            ]

    # u_top stream: [level][pair][node]
    node_idx = np.concatenate(
        [np.arange(2**i - 1, 2**i - 1 + 2**i) for i in range(10)]
    )  # [1023] heap indices, level-major
    # per level block repeated per pair
    tops = np.empty((18, UTOP_COLS), dtype=np.float32)
    colptr = 0
    cT = np.transpose(contents, (0, 2, 1))  # [16, 8, 16383]
    for i in range(10):
        off, n = 2**i - 1, 2**i
        for p in range(NPAIR):
            tops[0:8, colptr : colptr + n] = cT[2 * p][:, off : off + n]
            tops[8, colptr : colptr + n] = 1.0
            tops[9:17, colptr : colptr + n] = cT[2 * p + 1][:, off : off + n]
            tops[17, colptr : colptr + n] = 1.0
            colptr += n
    assert colptr == UTOP_COLS
    ctop = np.zeros((128, 2048), dtype=BF16)
    tb = np.zeros((18, UTOP_PAD), dtype=BF16)
    tb[:, :UTOP_COLS] = tops.astype(BF16)
    for g in range(8):
        t = g % 4
        cc = 1024 * (g // 4)
        ctop[32 * t : 32 * t + 18, cc : cc + 1024] = tb[:, 1024 * g : 1024 * (g + 1)]
    return {"c4": c4, "ctop": ctop}


def _prep_weights(Wu, bu, Wh, bh):
    wu2 = np.zeros((18, 128), dtype=np.float32)
    wu2[0:8, 0:64] = Wu
    wu2[8, 0:64] = bu
    wu2[9:17, 64:128] = Wu
    wu2[17, 64:128] = bu
    wu_dram = np.zeros((128, 128), dtype=BF16)
    for t in range(4):
        wu_dram[32 * t : 32 * t + 18, :] = wu2.astype(BF16)

    def blockdiag(Wx):
        out = np.zeros((128, 128), dtype=np.float32)
        out[0:64, 0:64] = Wx
        out[64:128, 64:128] = Wx
        return out.astype(BF16)

    whl = blockdiag(Wh[0:H])
    whr = blockdiag(Wh[H : 2 * H])
    whu = blockdiag(Wh[2 * H : 3 * H])
    bh2 = np.concatenate([bh, bh]).astype(np.float32).reshape(128, 1)
    return {"wu": wu_dram, "whl": whl, "whr": whr, "whu": whu, "bh2": bh2}


def _dedup_ldweights(nc):
    """Delete an LDWEIGHTS whose signature matches the immediately-preceding
    PE weight load (only MATMULs in between): the PE keeps the stationary
    operand resident, so load-once-matmul-many is safe. Sync info of deleted
    loads is merged into the following PE instruction."""
    n_del = 0
    for f in nc.m.functions:
        for bb in f.blocks:
            last_sig = None
            pending_sync = None
            out = []
            for inst in bb.instructions:
                tn = type(inst).__name__
                if str(getattr(inst, "engine", "")) == "EngineType.PE":
                    if tn == "InstLdweights":
                        a = inst.ins[0]
                        sig = (
                            getattr(a, "memref", None),
                            getattr(a, "offset", None),
                            str(getattr(a, "ap", None)),
                            str(inst.tile_position),
                            str(inst.tile_size),
                            str(inst.perf_mode),
                            str(inst.is_transpose),
                        )
                        if sig == last_sig:
                            n_del += 1
                            si = inst.sync_info
                            if si is not None and (si.on_wait or si.on_update):
                                if pending_sync is None:
                                    pending_sync = ([], [])
                                pending_sync[0].extend(si.on_wait)
                                pending_sync[1].extend(si.on_update)
                            continue  # drop this instruction
                        last_sig = sig
                    elif tn != "InstMatmult":
                        last_sig = None  # anything else on PE invalidates
                    if pending_sync is not None:
                        si = inst.sync_info
                        if si is None:
                            import concourse.mybir as mybir

                            inst.sync_info = mybir.SyncInfo(
                                on_wait=list(pending_sync[0]),
                                on_update=list(pending_sync[1]),
                            )
                        else:
                            si.on_wait[:0] = pending_sync[0]
                            si.on_update.extend(pending_sync[1])
                        pending_sync = None
                out.append(inst)
            assert pending_sync is None, "dangling sync from deleted trailing LDW"
            bb.instructions.clear()
            for i in out:
                bb.add_instruction(i)
    return n_del


def _split_sync_waits(nc, mybir, max_waits=1):
    """This container's walrus only accepts 1 sync-wait per instruction;
    move excess waits onto preceding same-engine NoOps."""
    for f in nc.m.functions:
        for bb in f.blocks:
            out = []
            for inst in bb.instructions:
                si = inst.sync_info
                if si is not None and len(si.on_wait) > max_waits:
                    waits = list(si.on_wait)
                    extra, keep = waits[:-max_waits], waits[-max_waits:]
                    for i in range(0, len(extra), max_waits):
                        nop = mybir.InstNoOp(
                            name=nc.get_next_instruction_name(),
                            engine=inst.engine,
                            sync_info=mybir.SyncInfo(
                                on_wait=extra[i : i + max_waits], on_update=[]
                            ),
                        )
                        out.append(nop)
                    si.on_wait = keep
                out.append(inst)
            bb.instructions.clear()
            for i in out:
                bb.add_instruction(i)


def _build_nc():
    import concourse.bass as bass
    import concourse.mybir as mybir
    from concourse.tile import TileContext

    fp32 = mybir.dt.float32
    bf16 = mybir.dt.bfloat16
    RELU = mybir.ActivationFunctionType.Relu
    ADD = mybir.AluOpType.add
    MAX = mybir.AluOpType.max

    nc = bass.Bass(trn_type="TRN2", num_devices=N_CORES)
    c4_d = nc.dram_tensor("c4", [NPAIR, 128, 4096], bf16, kind="ExternalInput")
    ctop_d = nc.dram_tensor("ctop", [128, 2048], bf16, kind="ExternalInput")
    wu_d = nc.dram_tensor("wu", [128, 128], bf16, kind="ExternalInput")
    whl_d = nc.dram_tensor("whl", [128, 128], bf16, kind="ExternalInput")
    whr_d = nc.dram_tensor("whr", [128, 128], bf16, kind="ExternalInput")
    whu_d = nc.dram_tensor("whu", [128, 128], bf16, kind="ExternalInput")
    bh2_d = nc.dram_tensor("bh2", [128, 1], fp32, kind="ExternalInput")
    out_d = nc.dram_tensor("out", [128, NPAIR], fp32, kind="ExternalOutput")

    act_tick = 0

    with TileContext(nc) as tc:
        with (
            tc.tile_pool(name="wpool", bufs=1) as wpool,
            tc.tile_pool(name="c4pool", bufs=3) as c4pool,
            tc.tile_pool(name="uspool", bufs=2) as uspool,
            tc.tile_pool(name="utpool", bufs=1) as utpool,
            tc.tile_pool(name="e12pool", bufs=2) as e12pool,
            tc.tile_pool(name="e11pool", bufs=2) as e11pool,
            tc.tile_pool(name="shpool", bufs=1) as shpool,
            tc.tile_pool(name="upsum", bufs=2, space="PSUM") as upsum,
            tc.tile_pool(name="lpsum", bufs=2, space="PSUM") as lpsum,
        ):
            wu_sb = wpool.tile([128, 128], bf16, tag="wu")
            whl_sb = wpool.tile([128, 128], bf16, tag="whl")
            whr_sb = wpool.tile([128, 128], bf16, tag="whr")
            whu_sb = wpool.tile([128, 128], bf16, tag="whu")
            bh_sb = wpool.tile([128, 1], fp32, tag="bh")
            ctop_sb = wpool.tile([128, 2048], bf16, tag="ctop")
            nc.sync.dma_start(wu_sb[:], wu_d.ap())
            nc.sync.dma_start(whl_sb[:], whl_d.ap())
            nc.sync.dma_start(whr_sb[:], whr_d.ap())
            nc.sync.dma_start(whu_sb[:], whu_d.ap())
            nc.sync.dma_start(bh_sb[:], bh2_d.ap())
            nc.sync.dma_start(ctop_sb[:], ctop_d.ap())

            def act_one(engine, dst_ap, src_ap, bias):
                if engine == 0:
                    if bias is None:
                        nc.scalar.activation(dst_ap, src_ap, RELU)
                    else:
                        nc.scalar.activation(dst_ap, src_ap, RELU, bias=bias)
                else:
                    if bias is None:
                        nc.vector.tensor_scalar(dst_ap, src_ap, 0.0, None, MAX)
                    else:
                        nc.vector.tensor_scalar(dst_ap, src_ap, bias, 0.0, ADD, MAX)

            def act_relu(dst_tile, dst_lo, ps_tile, w, bias):
                """relu(ps[0:w] + bias) -> dst[dst_lo:dst_lo+w], alternating
                ScalarE/VectorE per call."""
                nonlocal act_tick
                act_tick += 1
                act_one(
                    act_tick % 2,
                    dst_tile[:, dst_lo : dst_lo + w],
                    ps_tile[:, 0:w],
                    bias,
                )

            def u_phase(src_sb, ngroups, dst_tile):
                """fc_u matmuls+acts for one strip-packed input.
                Groups ordered strip-major so consecutive matmuls share the
                stationary operand (same PE row-group)."""
                order = [g for t in range(4) for g in range(ngroups) if g % 4 == t]
                for g in order:
                    t = g % 4
                    cc = 1024 * (g // 4)
                    ps = upsum.tile([128, 1024], fp32, tag="ups")
                    for h in range(2):
                        nc.tensor.matmul(
                            ps[:, 512 * h : 512 * (h + 1)],
                            wu_sb[32 * t : 32 * t + 18, :],
                            src_sb[
                                32 * t : 32 * t + 18,
                                cc + 512 * h : cc + 512 * (h + 1),
                            ],
                            start=True,
                            stop=True,
                            tile_position=(32 * t, 0),
                        )
                    act_relu(dst_tile, 1024 * g, ps, 1024, None)

            def levels_12_10(p, ustr, emb10sh):
                emb12 = e12pool.tile([128, 4096], bf16, tag="e12")
                emb11 = e11pool.tile([128, 2048], bf16, tag="e11")
                for i, ubase, prev, prev_base, dst, dst_base in (
                    (12, UB12, ustr, ULEAF, emb12, 0),
                    (11, UB11, emb12, 0, emb11, 0),
                    (10, UB10, emb11, 0, emb10sh, p * 1024),
                ):
                    m = 2**i
                    # process psum tiles in pairs, stationary-major, so the
                    # LDW-dedup pass collapses to one weight load per
                    # stationary per tile-pair
                    tiles = list(range(m // 1024))
                    for c0 in range(0, len(tiles), 2):
                        grp = tiles[c0 : c0 + 2]
                        pss = [
                            lpsum.tile(
                                [128, 1024], fp32, tag="lps", name=f"lps_{i}_{c0}_{ci}"
                            )
                            for ci in range(len(grp))
                        ]
                        for w_sb, src_of in (
                            (whl_sb, "L"),
                            (whr_sb, "R"),
                            (whu_sb, "U"),
                        ):
                            for ci, c in enumerate(grp):
                                for h in range(2):
                                    j0 = 1024 * c + 512 * h
                                    if src_of == "L":
                                        mv = prev[
                                            :,
                                            prev_base
                                            + 2 * j0 : prev_base
                                            + 2 * j0
                                            + 1024 : 2,
                                        ]
                                    elif src_of == "R":
                                        mv = prev[
                                            :,
                                            prev_base
                                            + 2 * j0
                                            + 1 : prev_base
                                            + 2 * j0
                                            + 1024 : 2,
                                        ]
                                    else:
                                        mv = ustr[:, ubase + j0 : ubase + j0 + 512]
                                    nc.tensor.matmul(
                                        pss[ci][:, 512 * h : 512 * (h + 1)],
                                        w_sb[:],
                                        mv,
                                        start=(src_of == "L"),
                                        stop=(src_of == "U"),
                                    )
                        for ci, c in enumerate(grp):
                            act_relu(dst, dst_base + 1024 * c, pss[ci], 1024, bh_sb[:])

            # ---- u_top: u for levels 0..9, all pairs ----
            utop = utpool.tile([128, UTOP_PAD], bf16, tag="utop")
            u_phase(ctop_sb, 8, utop)

            # ---- pairs, software-pipelined: u(p) emitted before levels(p-1)
            # so PE always has independent work while a level chain waits on
            # its activations ----
            emb10sh = shpool.tile([128, 8192], bf16, tag="e10")
            ustrs = [None] * NPAIR
            for p in range(NPAIR):
                c4_sb = c4pool.tile([128, 4096], bf16, tag="c4")
                nc.sync.dma_start(c4_sb[:], c4_d.ap()[p])
                ustrs[p] = uspool.tile([128, USTREAM], bf16, tag="us", name=f"ustr{p}")
                u_phase(c4_sb, NGRP, ustrs[p])
                if p > 0:
                    levels_12_10(p - 1, ustrs[p - 1], emb10sh)
            levels_12_10(NPAIR - 1, ustrs[NPAIR - 1], emb10sh)

            # ---- levels 9..1 batched over all pairs ----
            prev = emb10sh
            emb_sh = {}
            for i in range(9, 0, -1):
                m = 2**i
                M8 = 8 * m
                cur = shpool.tile([128, M8], bf16, tag=f"esh{i}")
                emb_sh[i] = cur
                base8 = 8 * (2**i - 1)
                starts = list(range(0, M8, 1024))
                for s0 in range(0, len(starts), 2):
                    grp = starts[s0 : s0 + 2]
                    pss = [
                        lpsum.tile(
                            [128, 1024], fp32, tag="lps", name=f"lpsb_{i}_{s0}_{ci}"
                        )
                        for ci in range(len(grp))
                    ]
                    for w_sb, kind in ((whl_sb, "L"), (whr_sb, "R"), (whu_sb, "U")):
                        for ci, c0 in enumerate(grp):
                            w = min(1024, M8 - c0)
                            for h0 in range(0, w, 512):
                                n = min(512, w - h0)
                                j0 = c0 + h0
                                if kind == "L":
                                    mv = prev[:, 2 * j0 : 2 * j0 + 2 * n : 2]
                                elif kind == "R":
                                    mv = prev[:, 2 * j0 + 1 : 2 * j0 + 2 * n : 2]
                                else:
                                    mv = utop[:, base8 + j0 : base8 + j0 + n]
                                nc.tensor.matmul(
                                    pss[ci][:, h0 : h0 + n],
                                    w_sb[:],
                                    mv,
                                    start=(kind == "L"),
                                    stop=(kind == "U"),
                                )
                    for ci, c0 in enumerate(grp):
                        act_relu(cur, c0, pss[ci], min(1024, M8 - c0), bh_sb[:])
                prev = cur

            # ---- level 0: root ----
            roots = wpool.tile([128, NPAIR], fp32, tag="roots")
            ps = lpsum.tile([128, 1024], fp32, tag="lps")
            o = ps[:, 0:NPAIR]
            nc.tensor.matmul(o, whl_sb[:], emb_sh[1][:, 0:16:2], start=True, stop=False)
            nc.tensor.matmul(o, whr_sb[:], emb_sh[1][:, 1:16:2], start=False, stop=False)
            nc.tensor.matmul(o, whu_sb[:], utop[:, 0:NPAIR], start=False, stop=True)
            nc.scalar.activation(roots[:], o, RELU, bias=bh_sb[:])
            nc.sync.dma_start(out_d.ap(), roots[:])

    _dedup_ldweights(nc)
    _split_sync_waits(nc, mybir)
    return nc


_NC_CACHE = None
LAST_RESULTS = None


def kernel(contents, children, Wu, bu, Wh, bh):
    global _NC_CACHE, LAST_RESULTS
    contents = np.asarray(contents, dtype=np.float32)
    children = np.asarray(children)
    Wu = np.asarray(Wu, dtype=np.float32)
    bu = np.asarray(bu, dtype=np.float32)
    Wh = np.asarray(Wh, dtype=np.float32)
    bh = np.asarray(bh, dtype=np.float32)

    regular = (
        contents.shape == (B, N_NODES, F)
        and children.shape == (N_INNER, 2)
        and np.array_equal(
            np.asarray(children, dtype=np.int64).ravel(), np.arange(N_INNER * 2)
        )
    )
    if not regular:
        # Safety net for non-arange children: exact numpy fallback.
        return _np_reference(contents, children, Wu, bu, Wh, bh)

    from concourse.bass_utils import run_bass_kernel_spmd

    if _NC_CACHE is None:
        _NC_CACHE = _build_nc()
    nc = _NC_CACHE

    wts = _prep_weights(Wu, bu, Wh, bh)
    in_maps = []
    for k in range(N_CORES):
        m = _prep_core_inputs(contents[JPC * k : JPC * (k + 1)])
        m.update(wts)
        in_maps.append(m)

    res = run_bass_kernel_spmd(
        nc,
        in_maps,
        core_ids=list(range(N_CORES)),
        trace=bool(os.environ.get("BASS_TRACE")),
    )
    LAST_RESULTS = res

    out = np.empty((B, H), dtype=np.float32)
    for k in range(N_CORES):
        r = res.results[k]["out"].reshape(2, 64, NPAIR)  # [half, h, pair]
        out[JPC * k : JPC * (k + 1)] = np.transpose(r, (2, 0, 1)).reshape(JPC, H)
    return out



# revision 2
# speedup vs baseline: 1.1267x; 1.1267x over previous
"""Trainium2 Bass kernel for GRNNTransformSimple (bottom-up binary-tree GRNN).

Computation (per jet): heap-layout complete binary tree, DEPTH=14.
  u_k   = relu(contents_k @ Wu + bu)                         (all nodes)
  emb_k = u_k                                                (leaves)
  emb_k = relu(hL @ Wh[:64] + hR @ Wh[64:128] + u_k @ Wh[128:] + bh)  (inner)
Output: root emb, [B, 64].

Mapping (8 NeuronCores, data-parallel over B=128 jets, 16 jets/core):
 - 2 jets packed per 128 SBUF partitions (jet A on partitions 0-63, jet B on
   64-127) with block-diagonal weights -> all engines run 128 partitions wide.
 - Deep levels (leaves + levels 12..10, 87% of the FLOPs) run in fp8e4m3
   with DoubleRow perf mode (2 K-planes per PE pass, 0.5 cyc per output col):
     * fc_u: planes (wu2, 0) with the moving contents broadcast across both
       planes (stride-0 plane dim) -> 2x faster than bf16.
     * fc_h L+R: planes (WhL_bd, WhR_bd) with moving planes (hL, hR) taken
       as the even/odd column interleave of the child level -> 4x faster.
     * fc_h U-term: planes (WhU_bd, 0) with broadcast u -> 2x faster.
   Deep-level fp8 quantization noise averages out on the way up the tree
   (measured: rel_rms 4.1e-3 vs 4.1e-3 pure-bf16 on the reference inputs);
   fp8 on the top levels does NOT average and is left in bf16.
 - Levels 9..0 are batched across pairs in bf16, split into two independent
   4-pair chains so one chain's matmuls fill the other's activation stalls.
 - fc_u biases folded into the matmul via a constant-one input row (K=18),
   with 4-way PE row tiling (tile_position) since K=18 << 128.
 - The "irregular" child gather is regular for arange children: children of
   level-i node j are nodes 2j, 2j+1 of level i+1, i.e. stride-2 column
   slices (the fp8 DoubleRow plane view) of the level-(i+1) embedding.
 - c4 (deep contents) is DMA'd in 128KB column chunks so the first matmul
   starts ~1us in; the u phase for levels 0..9 (utop) is emitted last to
   fill tail stalls.
 - relu activations are split between the Scalar and Vector engines by a
   greedy balance on estimated cost (the act engines are the critical
   resource in this regime).
"""

import os
import sys

sys.path.insert(0, "/opt/trn_rl_repo")

import ml_dtypes
import numpy as np

DEPTH = 14
B = 128
F = 8
H = 64
N_NODES = 2**DEPTH - 1  # 16383
N_INNER = 2 ** (DEPTH - 1) - 1  # 8191
N_CORES = 8
JPC = 16  # jets per core
NPAIR = 8  # jet pairs per core

BF16 = ml_dtypes.bfloat16
FP8 = ml_dtypes.float8_e4m3fn

# u_stream layout per pair (columns): levels 10,11,12 inner nodes in heap
# order, then all leaves in heap order.
UB10, UB11, UB12 = 0, 1024, 3072  # level bases inside u_stream
ULEAF = 7168
USTREAM = 15360  # 1024 + 2048 + 4096 + 8192
NGRP = 15  # 15 groups x 1024 cols
# u_top: levels 0..9, column order [level][pair][node]
UTOP_COLS = 8184  # 8 * 1023
UTOP_PAD = 8192


def _np_reference(contents, children, Wu, bu, Wh, bh):
    emb = None
    for i in range(DEPTH - 1, -1, -1):
        off, n = 2**i - 1, 2**i
        u = np.maximum(contents[:, off : off + n] @ Wu + bu, 0)
        if emb is None:
            emb = u
        else:
            ch = children[off : off + n] - 2 * off
            hL = emb[:, ch[:, 0]]
            hR = emb[:, ch[:, 1]]
            emb = np.maximum(
                hL @ Wh[:H] + hR @ Wh[H : 2 * H] + u @ Wh[2 * H :] + bh, 0
            )
    return emb.reshape(emb.shape[0], -1).astype(np.float32)


def _prep_core_inputs(contents):
    """contents: [16, 16383, 8] f32 for one core.
    Returns dict of per-core device input arrays."""
    c4 = np.zeros((NPAIR, 128, 4096), dtype=FP8)
    big_T = np.ascontiguousarray(
        np.transpose(contents[:, 1023:16383, :], (0, 2, 1))
    )  # [16, 8, 15360]
    for p in range(NPAIR):
        S = np.empty((18, USTREAM), dtype=np.float32)
        S[0:8] = big_T[2 * p]
        S[8] = 1.0
        S[9:17] = big_T[2 * p + 1]
        S[17] = 1.0
        S8 = S.astype(FP8)
        for g in range(NGRP):
            t = g % 4
            cc = 1024 * (g // 4)
            c4[p, 32 * t : 32 * t + 18, cc : cc + 1024] = S8[
                :, 1024 * g : 1024 * (g + 1)
            ]

    # u_top stream: [level][pair][node]
    tops = np.empty((18, UTOP_COLS), dtype=np.float32)
    colptr = 0
    cT = np.transpose(contents, (0, 2, 1))  # [16, 8, 16383]
    for i in range(10):
        off, n = 2**i - 1, 2**i
        for p in range(NPAIR):
            tops[0:8, colptr : colptr + n] = cT[2 * p][:, off : off + n]
            tops[8, colptr : colptr + n] = 1.0
            tops[9:17, colptr : colptr + n] = cT[2 * p + 1][:, off : off + n]
            tops[17, colptr : colptr + n] = 1.0
            colptr += n
    assert colptr == UTOP_COLS
    ctop = np.zeros((128, 2048), dtype=BF16)
    tb = np.zeros((18, UTOP_PAD), dtype=BF16)
    tb[:, :UTOP_COLS] = tops.astype(BF16)
    for g in range(8):
        t = g % 4
        cc = 1024 * (g // 4)
        ctop[32 * t : 32 * t + 18, cc : cc + 1024] = tb[:, 1024 * g : 1024 * (g + 1)]
    return {"c4": c4, "ctop": ctop}


def _prep_weights(Wu, bu, Wh, bh):
    wu2 = np.zeros((18, 128), dtype=np.float32)
    wu2[0:8, 0:64] = Wu
    wu2[8, 0:64] = bu
    wu2[9:17, 64:128] = Wu
    wu2[17, 64:128] = bu
    # fp8 DoubleRow stationary: plane 0 = wu2, plane 1 = 0, per 32-row strip
    wu8 = np.zeros((128, 256), dtype=FP8)
    for t in range(4):
        wu8[32 * t : 32 * t + 18, 0:128] = wu2.astype(FP8)
    # bf16 stationary for the top-levels u phase
    wub = np.zeros((128, 128), dtype=BF16)
    for t in range(4):
        wub[32 * t : 32 * t + 18, :] = wu2.astype(BF16)

    def blockdiag(Wx):
        out = np.zeros((128, 128), dtype=np.float32)
        out[0:64, 0:64] = Wx
        out[64:128, 64:128] = Wx
        return out

    bdl = blockdiag(Wh[0:H])
    bdr = blockdiag(Wh[H : 2 * H])
    bdu = blockdiag(Wh[2 * H : 3 * H])
    whlr8 = np.concatenate([bdl, bdr], axis=1).astype(FP8)
    whu8 = np.concatenate([bdu, np.zeros((128, 128), np.float32)], axis=1).astype(FP8)
    bh2 = np.concatenate([bh, bh]).astype(np.float32).reshape(128, 1)
    return {
        "wu8": wu8,
        "wub": wub,
        "whlr8": whlr8,
        "whu8": whu8,
        "whlb": bdl.astype(BF16),
        "whrb": bdr.astype(BF16),
        "whub": bdu.astype(BF16),
        "bh2": bh2,
    }


def _dedup_ldweights(nc):
    """Delete an LDWEIGHTS whose signature matches the immediately-preceding
    PE weight load (only MATMULs in between): the PE keeps the stationary
    operand resident, so load-once-matmul-many is safe. Sync info of deleted
    loads is merged into the following PE instruction."""
    n_del = 0
    for f in nc.m.functions:
        for bb in f.blocks:
            last_sig = None
            pending_sync = None
            out = []
            for inst in bb.instructions:
                tn = type(inst).__name__
                if str(getattr(inst, "engine", "")) == "EngineType.PE":
                    if tn == "InstLdweights":
                        a = inst.ins[0]
                        sig = (
                            getattr(a, "memref", None),
                            getattr(a, "offset", None),
                            str(getattr(a, "ap", None)),
                            str(inst.tile_position),
                            str(inst.tile_size),
                            str(inst.perf_mode),
                            str(inst.is_transpose),
                        )
                        if sig == last_sig:
                            n_del += 1
                            si = inst.sync_info
                            if si is not None and (si.on_wait or si.on_update):
                                if pending_sync is None:
                                    pending_sync = ([], [])
                                pending_sync[0].extend(si.on_wait)
                                pending_sync[1].extend(si.on_update)
                            continue  # drop this instruction
                        last_sig = sig
                    elif tn != "InstMatmult":
                        last_sig = None  # anything else on PE invalidates
                    if pending_sync is not None:
                        si = inst.sync_info
                        if si is None:
                            import concourse.mybir as mybir

                            inst.sync_info = mybir.SyncInfo(
                                on_wait=list(pending_sync[0]),
                                on_update=list(pending_sync[1]),
                            )
                        else:
                            si.on_wait[:0] = pending_sync[0]
                            si.on_update.extend(pending_sync[1])
                        pending_sync = None
                out.append(inst)
            assert pending_sync is None, "dangling sync from deleted trailing LDW"
            bb.instructions.clear()
            for i in out:
                bb.add_instruction(i)
    return n_del


def _split_sync_waits(nc, mybir, max_waits=1):
    """This container's walrus only accepts 1 sync-wait per instruction;
    move excess waits onto preceding same-engine NoOps."""
    for f in nc.m.functions:
        for bb in f.blocks:
            out = []
            for inst in bb.instructions:
                si = inst.sync_info
                if si is not None and len(si.on_wait) > max_waits:
                    waits = list(si.on_wait)
                    extra, keep = waits[:-max_waits], waits[-max_waits:]
                    for i in range(0, len(extra), max_waits):
                        nop = mybir.InstNoOp(
                            name=nc.get_next_instruction_name(),
                            engine=inst.engine,
                            sync_info=mybir.SyncInfo(
                                on_wait=extra[i : i + max_waits], on_update=[]
                            ),
                        )
                        out.append(nop)
                    si.on_wait = keep
                out.append(inst)
            bb.instructions.clear()
            for i in out:
                bb.add_instruction(i)


def _build_nc():
    import concourse.bass as bass
    import concourse.mybir as mybir
    from concourse.tile import TileContext

    fp32 = mybir.dt.float32
    bf16 = mybir.dt.bfloat16
    fp8 = mybir.dt.float8e4
    RELU = mybir.ActivationFunctionType.Relu
    ADD = mybir.AluOpType.add
    MAX = mybir.AluOpType.max
    DR = mybir.MatmulPerfMode.DoubleRow

    nc = bass.Bass(trn_type="TRN2", num_devices=N_CORES)
    c4_d = nc.dram_tensor("c4", [NPAIR, 128, 4096], fp8, kind="ExternalInput")
    ctop_d = nc.dram_tensor("ctop", [128, 2048], bf16, kind="ExternalInput")
    wu8_d = nc.dram_tensor("wu8", [128, 256], fp8, kind="ExternalInput")
    wub_d = nc.dram_tensor("wub", [128, 128], bf16, kind="ExternalInput")
    whlr8_d = nc.dram_tensor("whlr8", [128, 256], fp8, kind="ExternalInput")
    whu8_d = nc.dram_tensor("whu8", [128, 256], fp8, kind="ExternalInput")
    whlb_d = nc.dram_tensor("whlb", [128, 128], bf16, kind="ExternalInput")
    whrb_d = nc.dram_tensor("whrb", [128, 128], bf16, kind="ExternalInput")
    whub_d = nc.dram_tensor("whub", [128, 128], bf16, kind="ExternalInput")
    bh2_d = nc.dram_tensor("bh2", [128, 1], fp32, kind="ExternalInput")
    out_d = nc.dram_tensor("out", [128, NPAIR], fp32, kind="ExternalOutput")

    # greedy act-engine balance: [scalar(ACT), vector(DVE)] cumulative ns
    eng_load = [0.0, 0.0]

    with TileContext(nc) as tc:
        with (
            tc.tile_pool(name="wpool", bufs=1) as wpool,
            tc.tile_pool(name="c4pool", bufs=8) as c4pool,
            tc.tile_pool(name="uspool", bufs=2) as uspool,
            tc.tile_pool(name="e12pool", bufs=2) as e12pool,
            tc.tile_pool(name="e11pool", bufs=2) as e11pool,
            tc.tile_pool(name="shpool", bufs=1) as shpool,
            tc.tile_pool(name="pspool", bufs=4, space="PSUM") as pspool,
        ):
            wu8_sb = wpool.tile([128, 256], fp8, tag="wu8")
            whlr_sb = wpool.tile([128, 256], fp8, tag="whlr")
            whu8_sb = wpool.tile([128, 256], fp8, tag="whu8")
            bh_sb = wpool.tile([128, 1], fp32, tag="bh")
            wub_sb = wpool.tile([128, 128], bf16, tag="wub")
            whlb_sb = wpool.tile([128, 128], bf16, tag="whlb")
            whrb_sb = wpool.tile([128, 128], bf16, tag="whrb")
            whub_sb = wpool.tile([128, 128], bf16, tag="whub")
            ctop_sb = wpool.tile([128, 2048], bf16, tag="ctop")
            utop = wpool.tile([128, UTOP_PAD], bf16, tag="utop")
            # critical-path weights first
            nc.sync.dma_start(wu8_sb[:], wu8_d.ap())
            nc.sync.dma_start(whlr_sb[:], whlr8_d.ap())
            nc.sync.dma_start(whu8_sb[:], whu8_d.ap())
            nc.sync.dma_start(bh_sb[:], bh2_d.ap())

            whlr_v = whlr_sb[:, 0:256].rearrange("p (two m) -> p two m", two=2)
            whu_v = whu8_sb[:, 0:256].rearrange("p (two m) -> p two m", two=2)

            def act_relu(dst_ap, src_ap, bias, ncols):
                """relu(src + bias) -> dst on the act engine with the least
                estimated accumulated load."""
                cost = (230.0 + 0.833 * ncols, 147.0 + 1.042 * ncols)
                e = 0 if eng_load[0] + cost[0] <= eng_load[1] + cost[1] else 1
                eng_load[e] += cost[e]
                if e == 0:
                    if bias is None:
                        nc.scalar.activation(dst_ap, src_ap, RELU)
                    else:
                        nc.scalar.activation(dst_ap, src_ap, RELU, bias=bias)
                else:
                    if bias is None:
                        nc.vector.tensor_scalar(dst_ap, src_ap, 0.0, None, MAX)
                    else:
                        nc.vector.tensor_scalar(dst_ap, src_ap, bias, 0.0, ADD, MAX)

            def u_phase_fp8(p, chunks, ustr):
                """fc_u for the deep stream of one pair: fp8 DoubleRow with a
                broadcast (stride-0) second plane against zero weights."""
                for g in range(NGRP):
                    t, j = g % 4, g // 4
                    ch = chunks[j]
                    ps = pspool.tile([128, 1024], fp32, tag="ps", name=f"psu{p}_{g}")
                    wv = wu8_sb[32 * t : 32 * t + 18, :].rearrange(
                        "p (two m) -> p two m", two=2
                    )
                    cc = 1024 * j
                    for h in range(2):
                        mv = (
                            ch[32 * t : 32 * t + 18, 512 * h : 512 * (h + 1)]
                            .unsqueeze(1)
                            .broadcast_to([18, 2, 512])
                        )
                        nc.tensor.matmul(
                            ps[:, 512 * h : 512 * (h + 1)],
                            wv,
                            mv,
                            start=True,
                            stop=True,
                            perf_mode=DR,
                            tile_position=(32 * t, 0),
                        )
                    act_relu(
                        ustr[:, 1024 * g : 1024 * (g + 1)], ps[:, 0:1024], None, 1024
                    )

            def levels_deep(p, ustr, emb10sh):
                """fc_h levels 12..10 for one pair, fp8 DoubleRow."""
                emb12 = e12pool.tile([128, 4096], fp8, tag="e12")
                emb11 = e11pool.tile([128, 2048], fp8, tag="e11")
                for i, ubase, prev, prev_base, dst, dst_base in (
                    (12, UB12, ustr, ULEAF, emb12, 0),
                    (11, UB11, emb12, 0, emb11, 0),
                    (10, UB10, emb11, 0, emb10sh, 1024 * p),
                ):
                    m = 2**i
                    for c in range(m // 1024):
                        ps = pspool.tile(
                            [128, 1024], fp32, tag="ps", name=f"psl{p}_{i}_{c}"
                        )
                        for h in range(2):
                            j0 = 1024 * c + 512 * h
                            mv = prev[
                                :, prev_base + 2 * j0 : prev_base + 2 * j0 + 1024
                            ].rearrange("p (n two) -> p two n", two=2)
                            nc.tensor.matmul(
                                ps[:, 512 * h : 512 * (h + 1)],
                                whlr_v,
                                mv,
                                start=True,
                                stop=False,
                                perf_mode=DR,
                            )
                        for h in range(2):
                            j0 = 1024 * c + 512 * h
                            uv = (
                                ustr[:, ubase + j0 : ubase + j0 + 512]
                                .unsqueeze(1)
                                .broadcast_to([128, 2, 512])
                            )
                            nc.tensor.matmul(
                                ps[:, 512 * h : 512 * (h + 1)],
                                whu_v,
                                uv,
                                start=False,
                                stop=True,
                                perf_mode=DR,
                            )
                        act_relu(
                            dst[:, dst_base + 1024 * c : dst_base + 1024 * (c + 1)],
                            ps[:, 0:1024],
                            bh_sb[:],
                            1024,
                        )

            # ---- pairs, software-pipelined: u(p) emitted before levels(p-1) ----
            emb10sh = shpool.tile([128, 8192], bf16, tag="e10")
            ustrs = [None] * NPAIR
            chunked = [None] * NPAIR
            for p in range(NPAIR):
                chunks = []
                for j in range(4):
                    ch = c4pool.tile([128, 1024], fp8, tag="c4", name=f"c4_{p}_{j}")
                    nc.sync.dma_start(
                        ch[:], c4_d.ap()[p][:, 1024 * j : 1024 * (j + 1)]
                    )
                    chunks.append(ch)
                chunked[p] = chunks
                ustrs[p] = uspool.tile([128, USTREAM], fp8, tag="us", name=f"ustr{p}")
                u_phase_fp8(p, chunks, ustrs[p])
                if p == 0:
                    # tail-phase inputs; emitted after the critical first chunk
                    nc.sync.dma_start(wub_sb[:], wub_d.ap())
                    nc.sync.dma_start(whlb_sb[:], whlb_d.ap())
                    nc.sync.dma_start(whrb_sb[:], whrb_d.ap())
                    nc.sync.dma_start(whub_sb[:], whub_d.ap())
                    nc.sync.dma_start(ctop_sb[:], ctop_d.ap())
                if p > 0:
                    levels_deep(p - 1, ustrs[p - 1], emb10sh)
            levels_deep(NPAIR - 1, ustrs[NPAIR - 1], emb10sh)

            # ---- u for levels 0..9 (bf16), emitted late as tail filler ----
            for g in [0, 4, 1, 5, 2, 6, 3, 7]:
                t, cc = g % 4, 1024 * (g // 4)
                ps = pspool.tile([128, 1024], fp32, tag="ps", name=f"psut{g}")
                for h in range(2):
                    nc.tensor.matmul(
                        ps[:, 512 * h : 512 * (h + 1)],
                        wub_sb[32 * t : 32 * t + 18, :],
                        ctop_sb[32 * t : 32 * t + 18, cc + 512 * h : cc + 512 * (h + 1)],
                        start=True,
                        stop=True,
                        tile_position=(32 * t, 0),
                    )
                act_relu(utop[:, 1024 * g : 1024 * (g + 1)], ps[:, 0:1024], None, 1024)

            # ---- levels 9..1 in two independent 4-pair chains (bf16) ----
            eshs = [{}, {}]
            for i in range(9, 0, -1):
                m4 = 4 * (2**i)
                b8 = 8 * (2**i - 1)
                for X in range(2):
                    prev = emb10sh if i == 9 else eshs[X][i + 1]
                    pb = 4096 * X if i == 9 else 0
                    cur = wpool.tile([128, m4], bf16, tag=f"esh{X}_{i}")
                    eshs[X][i] = cur
                    for s0 in range(0, m4, 1024):
                        w = min(1024, m4 - s0)
                        ps = pspool.tile(
                            [128, 1024], fp32, tag="ps", name=f"pst{X}_{i}_{s0}"
                        )
                        for w_sb, kind in (
                            (whlb_sb, "L"),
                            (whrb_sb, "R"),
                            (whub_sb, "U"),
                        ):
                            for h0 in range(0, w, 512):
                                n = min(512, w - h0)
                                j0 = s0 + h0
                                if kind == "L":
                                    mv = prev[:, pb + 2 * j0 : pb + 2 * j0 + 2 * n : 2]
                                elif kind == "R":
                                    mv = prev[
                                        :, pb + 2 * j0 + 1 : pb + 2 * j0 + 2 * n : 2
                                    ]
                                else:
                                    mv = utop[
                                        :, b8 + m4 * X + j0 : b8 + m4 * X + j0 + n
                                    ]
                                nc.tensor.matmul(
                                    ps[:, h0 : h0 + n],
                                    w_sb[:],
                                    mv,
                                    start=(kind == "L"),
                                    stop=(kind == "U"),
                                )
                        act_relu(cur[:, s0 : s0 + w], ps[:, 0:w], bh_sb[:], w)

            # ---- level 0: roots, one per chain ----
            roots = wpool.tile([128, NPAIR], fp32, tag="roots")
            for X in range(2):
                ps = pspool.tile([128, 1024], fp32, tag="ps", name=f"psroot{X}")
                o = ps[:, 0:4]
                e1 = eshs[X][1]
                nc.tensor.matmul(o, whlb_sb[:], e1[:, 0:8:2], start=True, stop=False)
                nc.tensor.matmul(o, whrb_sb[:], e1[:, 1:8:2], start=False, stop=False)
                nc.tensor.matmul(
                    o, whub_sb[:], utop[:, 4 * X : 4 * X + 4], start=False, stop=True
                )
                nc.scalar.activation(
                    roots[:, 4 * X : 4 * X + 4], o, RELU, bias=bh_sb[:]
                )
            nc.sync.dma_start(out_d.ap(), roots[:])

    _dedup_ldweights(nc)
    _split_sync_waits(nc, mybir)
    return nc


_NC_CACHE = None
LAST_RESULTS = None


def kernel(contents, children, Wu, bu, Wh, bh):
    global _NC_CACHE, LAST_RESULTS
    contents = np.asarray(contents, dtype=np.float32)
    children = np.asarray(children)
    Wu = np.asarray(Wu, dtype=np.float32)
    bu = np.asarray(bu, dtype=np.float32)
    Wh = np.asarray(Wh, dtype=np.float32)
    bh = np.asarray(bh, dtype=np.float32)

    regular = (
        contents.shape == (B, N_NODES, F)
        and children.shape == (N_INNER, 2)
        and np.array_equal(
            np.asarray(children, dtype=np.int64).ravel(), np.arange(N_INNER * 2)
        )
    )
    if not regular:
        # Safety net for non-arange children: exact numpy fallback.
        return _np_reference(contents, children, Wu, bu, Wh, bh)

    from concourse.bass_utils import run_bass_kernel_spmd

    if _NC_CACHE is None:
        _NC_CACHE = _build_nc()
    nc = _NC_CACHE

    wts = _prep_weights(Wu, bu, Wh, bh)
    in_maps = []
    for k in range(N_CORES):
        m = _prep_core_inputs(contents[JPC * k : JPC * (k + 1)])
        m.update(wts)
        in_maps.append(m)

    res = run_bass_kernel_spmd(
        nc,
        in_maps,
        core_ids=list(range(N_CORES)),
        trace=bool(os.environ.get("BASS_TRACE")),
    )
    LAST_RESULTS = res

    out = np.empty((B, H), dtype=np.float32)
    for k in range(N_CORES):
        r = res.results[k]["out"].reshape(2, 64, NPAIR)  # [half, h, pair]
        out[JPC * k : JPC * (k + 1)] = np.transpose(r, (2, 0, 1)).reshape(JPC, H)
    return out
